# revision 1
# baseline (speedup 1.0000x reference)
"""Decoder block kernel for trn2, 8 cores.

Sharding: core c -> batch b=c//4, token chunk o=c%4 (512 tokens of 2048).
Per-core program (SPMD uniform; per-core differences are data only):
  LN1 on own 512 tokens -> h; transpose -> h^T
  qkv^T = W_qkv-chunks.T @ h^T  (fp32r); k^T,v^T -> DRAM; AllGather(group of 4)
  attention: own 512 queries vs all 2048 keys (dense, additive mask data)
    scores^T[k,q] in PSUM, +mask, exp (ACT, scale=1/8), AV via v_aug(ones row)
    normalization by broadcasted reciprocal of the ones-row sums
  attn_proj + bias + residual -> x2^T; LN2 (via transpose to natural) -> h2^T
  MLP fc+bias+gelu_tanh (fused on ACT) -> g^T; proj + bias + residual -> y^T
  transpose -> y natural [512, 1024] -> DRAM out
"""
import sys

sys.path.insert(0, "/opt/trn_rl_repo")

import numpy as np
import concourse.bass as bass
import concourse.bacc as bacc
import concourse.mybir as mybir
import concourse.tile as tile
from concourse.masks import make_identity

f32 = mybir.dt.float32
f32r = mybir.dt.float32r

N_CORES = 8
GROUPS = [[0, 1, 2, 3], [4, 5, 6, 7]]
TOK = 512        # own tokens per core
E = 1024
HEADS = 16
DH = 64
HID = 4096
KEYS = 2048      # keys per batch
NKB = KEYS // 128   # 16 key blocks
NIC = E // 128      # 8 input chunks
NEG = -30000.0
AF = mybir.ActivationFunctionType
ALU = mybir.AluOpType


def build():
    nc = bacc.Bacc("TRN2", target_bir_lowering=False, num_devices=N_CORES)

    x_in = nc.dram_tensor("x", [TOK, E], f32, kind="ExternalInput")
    maskT = nc.dram_tensor("maskT", [KEYS, TOK], f32, kind="ExternalInput")
    w_qkv = nc.dram_tensor("w_qkv", [E, 3 * E], f32r, kind="ExternalInput")
    w_ap = nc.dram_tensor("w_ap", [E, E], f32r, kind="ExternalInput")
    w_fc = nc.dram_tensor("w_fc", [E, HID], f32r, kind="ExternalInput")
    w_pr = nc.dram_tensor("w_pr", [HID, E], f32r, kind="ExternalInput")
    b_ap = nc.dram_tensor("b_ap", [128, 8], f32, kind="ExternalInput")
    b_fc = nc.dram_tensor("b_fc", [128, 32], f32, kind="ExternalInput")
    b_pr = nc.dram_tensor("b_pr", [128, 8], f32, kind="ExternalInput")
    ln1w = nc.dram_tensor("ln1w", [E], f32, kind="ExternalInput")
    ln1b = nc.dram_tensor("ln1b", [E], f32, kind="ExternalInput")
    ln2w = nc.dram_tensor("ln2w", [E], f32, kind="ExternalInput")
    ln2b = nc.dram_tensor("ln2b", [E], f32, kind="ExternalInput")
    y_out = nc.dram_tensor("y", [TOK, E], f32, kind="ExternalOutput")

    def bcast(src, parts, free):
        return bass.AP(tensor=src.tensor if isinstance(src, bass.AP) else src,
                       offset=0, ap=[[0, parts], [1, free]])

    with tile.TileContext(nc) as tc:
        consts = tc.alloc_tile_pool(name="consts", bufs=1)
        acts = tc.alloc_tile_pool(name="acts", bufs=1)
        dram = tc.alloc_tile_pool(name="dram", bufs=1, space="DRAM")

        # ---- constants ----
        ident = consts.tile([128, 128], f32)
        make_identity(nc, ident[:, :])
        identr = consts.tile([128, 128], f32r)
        nc.vector.tensor_copy(identr[:, :], ident[:, :])
        eps_t = consts.tile([128, 1], f32)
        nc.vector.memset(eps_t[:, :], 1e-5)
        lnw_t = [consts.tile([128, E], f32, tag=f"lnw{i}") for i in range(2)]
        lnb_t = [consts.tile([128, E], f32, tag=f"lnb{i}") for i in range(2)]
        nc.sync.dma_start(out=lnw_t[0][:, :], in_=bcast(ln1w[:], 128, E))
        nc.sync.dma_start(out=lnb_t[0][:, :], in_=bcast(ln1b[:], 128, E))
        nc.sync.dma_start(out=lnw_t[1][:, :], in_=bcast(ln2w[:], 128, E))
        nc.sync.dma_start(out=lnb_t[1][:, :], in_=bcast(ln2b[:], 128, E))
        bap_t = consts.tile([128, 8], f32)
        bfc_t = consts.tile([128, 32], f32)
        bpr_t = consts.tile([128, 8], f32)
        nc.sync.dma_start(out=bap_t[:, :], in_=b_ap[:, :])
        nc.sync.dma_start(out=bfc_t[:, :], in_=b_fc[:, :])
        nc.sync.dma_start(out=bpr_t[:, :], in_=b_pr[:, :])

        # ---- persistent activation tiles ----
        xT = [acts.tile([128, TOK], f32, tag=f"xT{i}") for i in range(NIC)]
        hT = [acts.tile([128, TOK], f32r, tag=f"hT{i}") for i in range(NIC)]
        qT = [acts.tile([128, TOK], f32r, tag=f"qT{i}") for i in range(NIC)]
        ctxT = [acts.tile([128, TOK], f32r, tag=f"cT{i}") for i in range(NIC)]
        x2T = [acts.tile([128, TOK], f32, tag=f"x2T{i}") for i in range(NIC)]
        h2T = [acts.tile([128, TOK], f32r, tag=f"h2T{i}") for i in range(NIC)]
        yT = [acts.tile([128, TOK], f32, tag=f"yT{i}") for i in range(NIC)]

        kv_in = dram.tile([2 * E, TOK], f32r)
        kv_all = dram.tile([4 * 2 * E, TOK], f32r)

        # ================= LN1 (natural) + transposes =================
        def layernorm_natural(x_nat_tiles, w_tile, b_tile, out_pool, out_tag):
            """x_nat_tiles: 4x [128, E] f32 natural. Returns 4 normalized tiles."""
            outs = []
            for t in range(4):
                xt = x_nat_tiles[t]
                stats = out_pool.tile([128, 2, 6], f32, tag=f"{out_tag}_st")
                nc.vector.bn_stats(out=stats[:, 0, :], in_=xt[:, 0:512])
                nc.vector.bn_stats(out=stats[:, 1, :], in_=xt[:, 512:1024])
                mv = out_pool.tile([128, 2], f32, tag=f"{out_tag}_mv")
                nc.vector.bn_aggr(out=mv[:, :], in_=stats[:, :, :])
                rstd = out_pool.tile([128, 1], f32, tag=f"{out_tag}_rs")
                nc.scalar.activation(out=rstd[:, :], in_=mv[:, 1:2],
                                     func=AF.Sqrt, bias=eps_t[:, :], scale=1.0)
                nc.vector.reciprocal(out=rstd[:, :], in_=rstd[:, :])
                hn = out_pool.tile([128, E], f32, tag=f"{out_tag}_h")
                # (x - mean) * rstd
                nc.vector.tensor_scalar(hn[:, :], xt[:, :], mv[:, 0:1], rstd[:, :],
                                        ALU.subtract, ALU.mult)
                # * w + b
                nc.vector.scalar_tensor_tensor(
                    out=hn[:, :], in0=hn[:, :], scalar=1.0, in1=w_tile[:, :],
                    op0=ALU.bypass, op1=ALU.mult)
                nc.vector.tensor_add(hn[:, :], hn[:, :], b_tile[:, :])
                outs.append(hn)
            return outs

        with (
            tc.tile_pool(name="p1sb", bufs=2) as p1sb,
            tc.tile_pool(name="p1ps", bufs=4, space="PSUM") as p1ps,
        ):
            x_nat = []
            for t in range(4):
                xt = p1sb.tile([128, E], f32, tag="xnat")
                nc.sync.dma_start(out=xt[:, :], in_=x_in[128 * t:128 * (t + 1), :])
                x_nat.append(xt)
            h_nat = layernorm_natural(x_nat, lnw_t[0], lnb_t[0], p1sb, "ln1")
            # transposes: x^T (f32) and h^T (f32r)
            for t in range(4):
                for ic in range(NIC):
                    pt = p1ps.tile([128, 128], f32, tag="tp")
                    nc.tensor.transpose(pt[:, :], x_nat[t][:, 128 * ic:128 * (ic + 1)],
                                        ident[:, :])
                    nc.scalar.copy(out=xT[ic][:, 128 * t:128 * (t + 1)], in_=pt[:, :])
                    pt2 = p1ps.tile([128, 128], f32, tag="tp2")
                    nc.tensor.transpose(pt2[:, :], h_nat[t][:, 128 * ic:128 * (ic + 1)],
                                        ident[:, :])
                    nc.scalar.copy(out=hT[ic][:, 128 * t:128 * (t + 1)], in_=pt2[:, :])

        # ================= qkv^T (24 feature chunks x 8 in-chunks) =========
        with (
            tc.tile_pool(name="p2w", bufs=3) as p2w,
            tc.tile_pool(name="p2sb", bufs=3) as p2sb,
            tc.tile_pool(name="p2ps", bufs=2, space="PSUM") as p2ps,
        ):
            for g in range(6):  # groups of 4 feature chunks
                psums = [p2ps.tile([128, TOK], f32, tag=f"mm{j}") for j in range(4)]
                for ic in range(NIC):
                    wt = p2w.tile([128, 512], f32r, tag="w")
                    nc.sync.dma_start(
                        out=wt[:, :],
                        in_=w_qkv[128 * ic:128 * (ic + 1), 512 * g:512 * (g + 1)])
                    for j in range(4):
                        nc.tensor.matmul(psums[j][:, :], wt[:, 128 * j:128 * (j + 1)],
                                         hT[ic][:, :], start=(ic == 0),
                                         stop=(ic == NIC - 1))
                for j in range(4):
                    fc = 4 * g + j
                    if fc < 8:  # q rows -> resident qT
                        nc.scalar.copy(out=qT[fc][:, :], in_=psums[j][:, :])
                    else:       # k,v rows -> staging, then DRAM for AllGather
                        st = p2sb.tile([128, TOK], f32r, tag="kv")
                        nc.scalar.copy(out=st[:, :], in_=psums[j][:, :])
                        r0 = 128 * (fc - 8)
                        nc.sync.dma_start(out=kv_in[r0:r0 + 128, :], in_=st[:, :])

        # ================= AllGather k,v =================
        nc.gpsimd.collective_compute(
            "AllGather", ALU.bypass, replica_groups=GROUPS,
            ins=[kv_in.opt()], outs=[kv_all.opt()])

        # ================= attention =================
        with (
            tc.tile_pool(name="p3m", bufs=1) as p3m,
            tc.tile_pool(name="p3k", bufs=3) as p3k,
            tc.tile_pool(name="p3p", bufs=2) as p3p,
            tc.tile_pool(name="p3ps", bufs=1, space="PSUM") as p3ctx,
            tc.tile_pool(name="p3ss", bufs=2, space="PSUM") as p3ss,
            tc.tile_pool(name="p3tp", bufs=2, space="PSUM") as p3tp,
        ):
            mask_sb = p3m.tile([128, NKB * TOK], f32)
            for kb in range(NKB):
                nc.sync.dma_start(out=mask_sb[:, TOK * kb:TOK * (kb + 1)],
                                  in_=maskT[128 * kb:128 * (kb + 1), :])
            for hp in range(8):  # head pairs
                cpsA = p3ctx.tile([66, TOK], f32, tag="ctxA")
                cpsB = p3ctx.tile([66, TOK], f32, tag="ctxB")
                for kb in range(NKB):
                    j, ksl = kb // 4, kb % 4
                    kt = p3k.tile([128, 128], f32r, tag="kt")
                    nc.sync.dma_start(
                        out=kt[:, :],
                        in_=kv_all[2048 * j + 128 * hp:2048 * j + 128 * (hp + 1),
                                   128 * ksl:128 * (ksl + 1)])
                    msl = mask_sb[:, TOK * kb:TOK * (kb + 1)]
                    for h in range(2):  # head within pair
                        lo, hi = 64 * h, 64 * (h + 1)
                        sps = p3ss.tile([128, TOK], f32, tag=f"s{h}")
                        nc.tensor.matmul(sps[:, :], kt[lo:hi, :], qT[hp][lo:hi, :],
                                         start=True, stop=True,
                                         tile_position=(lo, 0))
                        nc.vector.tensor_add(sps[:, :], sps[:, :], msl)
                        pT = p3p.tile([128, TOK], f32r, tag=f"p{h}")
                        nc.scalar.activation(out=pT[:, :], in_=sps[:, :],
                                             func=AF.Exp, scale=0.125)
                        # v_aug: [128 keys, 66] with ones in col 64 (65 pad)
                        vt = p3k.tile([64, 128], f32r, tag=f"vt{h}")
                        r0 = 2048 * j + 1024 + 64 * (2 * hp + h)
                        nc.sync.dma_start(
                            out=vt[:, :],
                            in_=kv_all[r0:r0 + 64, 128 * ksl:128 * (ksl + 1)])
                        tp = p3tp.tile([128, 64], f32r, tag=f"tp{h}")
                        nc.tensor.transpose(tp[:, :], vt[:, :], identr[0:64, 0:64])
                        va = p3k.tile([128, 66], f32r, tag=f"va{h}")
                        nc.scalar.copy(out=va[:, 0:64], in_=tp[:, :])
                        nc.vector.memset(va[:, 64:66], 1.0)
                        cps = cpsA if h == 0 else cpsB
                        nc.tensor.matmul(cps[:, :], va[:, :], pT[:, :],
                                         start=(kb == 0), stop=(kb == NKB - 1))
                # normalize: rows 0:64 / row 64, write into ctxT[hp].
                # partition-broadcast of 1/sums via K=1 matmul with a ones col.
                for h, cps in ((0, cpsA), (1, cpsB)):
                    rec = p3p.tile([1, TOK], f32, tag=f"r{h}")
                    nc.vector.reciprocal(out=rec[:, :], in_=cps[64:65, :])
                    rec_r = p3p.tile([1, TOK], f32r, tag=f"rr{h}")
                    nc.scalar.copy(out=rec_r[:, :], in_=rec[:, :])
                    rbp = p3tp.tile([64, TOK], f32, tag=f"rb{h}")
                    nc.tensor.matmul(rbp[:, :], ones_r[:, :], rec_r[:, :],
                                     start=True, stop=True)
                    rb = p3p.tile([64, TOK], f32, tag=f"rbs{h}")
                    nc.scalar.copy(out=rb[:, :], in_=rbp[:, :])
                    nc.vector.tensor_tensor(
                        out=ctxT[hp][64 * h:64 * (h + 1), :],
                        in0=cps[0:64, :], in1=rb[:, :], op=ALU.mult)

        # ================= attn_proj + bias + residual =================
        with (
            tc.tile_pool(name="p4w", bufs=3) as p4w,
            tc.tile_pool(name="p4ps", bufs=2, space="PSUM") as p4ps,
        ):
            for g in range(2):
                psums = [p4ps.tile([128, TOK], f32, tag=f"mm{j}") for j in range(4)]
                for ic in range(NIC):
                    wt = p4w.tile([128, 512], f32r, tag="w")
                    nc.sync.dma_start(
                        out=wt[:, :],
                        in_=w_ap[128 * ic:128 * (ic + 1), 512 * g:512 * (g + 1)])
                    for j in range(4):
                        nc.tensor.matmul(psums[j][:, :], wt[:, 128 * j:128 * (j + 1)],
                                         ctxT[ic][:, :], start=(ic == 0),
                                         stop=(ic == NIC - 1))
                for j in range(4):
                    oc = 4 * g + j
                    # x2^T = (psum + b_ap[oc]) + x^T
                    nc.vector.scalar_tensor_tensor(
                        out=x2T[oc][:, :], in0=psums[j][:, :],
                        scalar=bap_t[:, oc:oc + 1], in1=xT[oc][:, :],
                        op0=ALU.add, op1=ALU.add)

        # ================= LN2 (transpose -> natural -> back) ==========
        with (
            tc.tile_pool(name="p5sb", bufs=2) as p5sb,
            tc.tile_pool(name="p5ps", bufs=4, space="PSUM") as p5ps,
        ):
            x2_nat = []
            for t in range(4):
                xt = p5sb.tile([128, E], f32, tag="x2nat")
                for ic in range(NIC):
                    pt = p5ps.tile([128, 128], f32, tag="tp")
                    nc.tensor.transpose(pt[:, :], x2T[ic][:, 128 * t:128 * (t + 1)],
                                        ident[:, :])
                    nc.scalar.copy(out=xt[:, 128 * ic:128 * (ic + 1)], in_=pt[:, :])
                x2_nat.append(xt)
            h2_nat = layernorm_natural(x2_nat, lnw_t[1], lnb_t[1], p5sb, "ln2")
            for t in range(4):
                for ic in range(NIC):
                    pt = p5ps.tile([128, 128], f32, tag="tp2")
                    nc.tensor.transpose(pt[:, :], h2_nat[t][:, 128 * ic:128 * (ic + 1)],
                                        ident[:, :])
                    nc.scalar.copy(out=h2T[ic][:, 128 * t:128 * (t + 1)], in_=pt[:, :])

        # ================= MLP =================
        with (
            tc.tile_pool(name="p6w", bufs=3) as p6w,
            tc.tile_pool(name="p6ps", bufs=2, space="PSUM") as p6ps,
        ):
            # reuse dead activation tag slots (hT/qT/cT/xT) for the 8MB g^T
            g_tags = ([f"hT{i}" for i in range(8)] + [f"qT{i}" for i in range(8)]
                      + [f"cT{i}" for i in range(8)] + [f"xT{i}" for i in range(8)])
            gT = [acts.tile([128, TOK], f32r, tag=g_tags[i]) for i in range(32)]
            for g in range(8):  # fc: groups of 4 hidden chunks
                psums = [p6ps.tile([128, TOK], f32, tag=f"mm{j}") for j in range(4)]
                for ic in range(NIC):
                    wt = p6w.tile([128, 512], f32r, tag="w")
                    nc.sync.dma_start(
                        out=wt[:, :],
                        in_=w_fc[128 * ic:128 * (ic + 1), 512 * g:512 * (g + 1)])
                    for j in range(4):
                        nc.tensor.matmul(psums[j][:, :], wt[:, 128 * j:128 * (j + 1)],
                                         h2T[ic][:, :], start=(ic == 0),
                                         stop=(ic == NIC - 1))
                for j in range(4):
                    hc = 4 * g + j
                    nc.scalar.activation(out=gT[hc][:, :], in_=psums[j][:, :],
                                         func=AF.Gelu_apprx_tanh,
                                         bias=bfc_t[:, hc:hc + 1], scale=1.0)
            for g in range(2):  # proj
                psums = [p6ps.tile([128, TOK], f32, tag=f"pm{j}") for j in range(4)]
                for ic in range(32):
                    wt = p6w.tile([128, 512], f32r, tag="w2")
                    nc.sync.dma_start(
                        out=wt[:, :],
                        in_=w_pr[128 * ic:128 * (ic + 1), 512 * g:512 * (g + 1)])
                    for j in range(4):
                        nc.tensor.matmul(psums[j][:, :], wt[:, 128 * j:128 * (j + 1)],
                                         gT[ic][:, :], start=(ic == 0),
                                         stop=(ic == 31))
                for j in range(4):
                    oc = 4 * g + j
                    nc.vector.scalar_tensor_tensor(
                        out=yT[oc][:, :], in0=psums[j][:, :],
                        scalar=bpr_t[:, oc:oc + 1], in1=x2T[oc][:, :],
                        op0=ALU.add, op1=ALU.add)

        # ================= output transpose + store =================
        with (
            tc.tile_pool(name="p7sb", bufs=2) as p7sb,
            tc.tile_pool(name="p7ps", bufs=4, space="PSUM") as p7ps,
        ):
            for t in range(4):
                yt = p7sb.tile([128, E], f32, tag="ynat")
                for ic in range(NIC):
                    pt = p7ps.tile([128, 128], f32, tag="tp")
                    nc.tensor.transpose(pt[:, :], yT[ic][:, 128 * t:128 * (t + 1)],
                                        ident[:, :])
                    nc.scalar.copy(out=yt[:, 128 * ic:128 * (ic + 1)], in_=pt[:, :])
                nc.sync.dma_start(out=y_out[128 * t:128 * (t + 1), :], in_=yt[:, :])

        dram.release()
        acts.release()
        consts.release()

    nc.compile()
    return nc


def make_in_maps(inputs):
    """inputs: the reference setup_inputs() dict. Returns list of 8 per-core maps."""
    x = np.asarray(inputs["x"], np.float32)
    w_qkv = np.asarray(inputs["w_qkv"], np.float32)
    w_ap = np.asarray(inputs["w_attn_proj"], np.float32)
    w_fc = np.asarray(inputs["w_fc"], np.float32)
    w_pr = np.asarray(inputs["w_proj"], np.float32)
    b_ap = np.asarray(inputs["b_attn_proj"], np.float32).reshape(8, 128).T.copy()
    b_fc = np.asarray(inputs["b_fc"], np.float32).reshape(32, 128).T.copy()
    b_pr = np.asarray(inputs["b_proj"], np.float32).reshape(8, 128).T.copy()
    ln1w = np.asarray(inputs["ln1_w"], np.float32)
    ln1b = np.asarray(inputs["ln1_b"], np.float32)
    ln2w = np.asarray(inputs["ln2_w"], np.float32)
    ln2b = np.asarray(inputs["ln2_b"], np.float32)

    in_maps = []
    kpos = np.arange(KEYS)[:, None]
    for c in range(N_CORES):
        b, o = c // 4, c % 4
        qpos = (TOK * o + np.arange(TOK))[None, :]
        mask = np.where(qpos >= kpos, 0.0, NEG).astype(np.float32)
        in_maps.append({
            "x": x[b, TOK * o:TOK * (o + 1), :],
            "maskT": mask,
            "w_qkv": w_qkv, "w_ap": w_ap, "w_fc": w_fc, "w_pr": w_pr,
            "b_ap": b_ap, "b_fc": b_fc, "b_pr": b_pr,
            "ln1w": ln1w, "ln1b": ln1b, "ln2w": ln2w, "ln2b": ln2b,
        })
    return in_maps


def assemble(results):
    y = np.zeros((2, KEYS, E), np.float32)
    for c in range(N_CORES):
        b, o = c // 4, c % 4
        y[b, TOK * o:TOK * (o + 1), :] = results[c]["y"]
    return y


# ======================================================================
# Harness entry point: full inputs -> full output, 8-core SPMD inside.
# ======================================================================
_NC_CACHE = {}


def kernel(**inputs):
    """Decoder block on 8 trn2 NeuronCores.

    Takes the full (unsharded) reference inputs, returns [2, 2048, 1024] f32.
    """
    from concourse.bass_utils import run_bass_kernel_spmd

    if "nc" not in _NC_CACHE:
        _NC_CACHE["nc"] = build(gelu_mode="hw")
    nc = _NC_CACHE["nc"]
    in_maps = make_in_maps(inputs)

    last_err = None
    for _attempt in range(3):
        try:
            res = run_bass_kernel_spmd(nc, in_maps, list(range(N_CORES)))
            return assemble([res.results[i] for i in range(N_CORES)])
        except Exception as e:  # wedged-device flake: retry
            last_err = e
    raise last_err


# revision 2
# speedup vs baseline: 1.0793x; 1.0793x over previous
"""Decoder block kernel for trn2, 8 cores.

Sharding: core c -> batch b=c//4, token chunk o=c%4 (512 tokens of 2048).
Per-core program (SPMD uniform; per-core differences are data only):
  LN1 on own 512 tokens -> h; transpose -> h^T
  qkv^T = W_qkv-chunks.T @ h^T  (fp32r); k^T,v^T -> DRAM; AllGather(group of 4)
  attention: own 512 queries vs all 2048 keys (dense, additive mask data)
    scores^T[k,q] in PSUM, +mask, exp (ACT, scale=1/8), AV via v_aug(ones row)
    normalization by broadcasted reciprocal of the ones-row sums
  attn_proj + bias + residual -> x2^T; LN2 (via transpose to natural) -> h2^T
  MLP fc+bias+gelu_tanh (fused on ACT) -> g^T; proj + bias + residual -> y^T
  transpose -> y natural [512, 1024] -> DRAM out
"""
import sys

sys.path.insert(0, "/opt/trn_rl_repo")

import numpy as np
import concourse.bass as bass
import concourse.bacc as bacc
import concourse.mybir as mybir
import concourse.tile as tile
from concourse.masks import make_identity

f32 = mybir.dt.float32
f32r = mybir.dt.float32r

N_CORES = 8
GROUPS = [[0, 1, 2, 3], [4, 5, 6, 7]]
TOK = 512        # own tokens per core
E = 1024
HEADS = 16
DH = 64
HID = 4096
KEYS = 2048      # keys per batch
NKB = KEYS // 128   # 16 key blocks
NIC = E // 128      # 8 input chunks
NEG = -30000.0
AF = mybir.ActivationFunctionType
ALU = mybir.AluOpType


def build():
    nc = bacc.Bacc("TRN2", target_bir_lowering=False, num_devices=N_CORES)

    x_in = nc.dram_tensor("x", [TOK, E], f32, kind="ExternalInput")
    maskT = nc.dram_tensor("maskT", [KEYS, TOK], f32, kind="ExternalInput")
    w_qkv = nc.dram_tensor("w_qkv", [E, 3 * E], f32r, kind="ExternalInput")
    w_ap = nc.dram_tensor("w_ap", [E, E], f32r, kind="ExternalInput")
    w_fc = nc.dram_tensor("w_fc", [E, HID], f32r, kind="ExternalInput")
    w_pr = nc.dram_tensor("w_pr", [HID, E], f32r, kind="ExternalInput")
    b_ap = nc.dram_tensor("b_ap", [128, 8], f32, kind="ExternalInput")
    b_fc = nc.dram_tensor("b_fc", [128, 32], f32, kind="ExternalInput")
    b_pr = nc.dram_tensor("b_pr", [128, 8], f32, kind="ExternalInput")
    ln1w = nc.dram_tensor("ln1w", [E], f32, kind="ExternalInput")
    ln1b = nc.dram_tensor("ln1b", [E], f32, kind="ExternalInput")
    ln2w = nc.dram_tensor("ln2w", [E], f32, kind="ExternalInput")
    ln2b = nc.dram_tensor("ln2b", [E], f32, kind="ExternalInput")
    y_out = nc.dram_tensor("y", [TOK, E], f32, kind="ExternalOutput")

    def bcast(src, parts, free):
        return bass.AP(tensor=src.tensor if isinstance(src, bass.AP) else src,
                       offset=0, ap=[[0, parts], [1, free]])

    with tile.TileContext(nc) as tc:
        consts = tc.alloc_tile_pool(name="consts", bufs=1)
        acts = tc.alloc_tile_pool(name="acts", bufs=1)
        dram = tc.alloc_tile_pool(name="dram", bufs=1, space="DRAM")

        # ---- constants ----
        ident = consts.tile([128, 128], f32)
        make_identity(nc, ident[:, :])
        identr = consts.tile([128, 128], f32r)
        nc.vector.tensor_copy(identr[:, :], ident[:, :])
        eps_t = consts.tile([128, 1], f32)
        nc.vector.memset(eps_t[:, :], 1e-5)
        lnw_t = [consts.tile([128, E], f32, tag=f"lnw{i}") for i in range(2)]
        lnb_t = [consts.tile([128, E], f32, tag=f"lnb{i}") for i in range(2)]
        nc.sync.dma_start(out=lnw_t[0][:, :], in_=bcast(ln1w[:], 128, E))
        nc.sync.dma_start(out=lnb_t[0][:, :], in_=bcast(ln1b[:], 128, E))
        nc.sync.dma_start(out=lnw_t[1][:, :], in_=bcast(ln2w[:], 128, E))
        nc.sync.dma_start(out=lnb_t[1][:, :], in_=bcast(ln2b[:], 128, E))
        bap_t = consts.tile([128, 8], f32)
        bfc_t = consts.tile([128, 32], f32)
        bpr_t = consts.tile([128, 8], f32)
        nc.sync.dma_start(out=bap_t[:, :], in_=b_ap[:, :])
        nc.sync.dma_start(out=bfc_t[:, :], in_=b_fc[:, :])
        nc.sync.dma_start(out=bpr_t[:, :], in_=b_pr[:, :])

        # ---- persistent activation tiles ----
        xT = [acts.tile([128, TOK], f32, tag=f"xT{i}") for i in range(NIC)]
        hT = [acts.tile([128, TOK], f32r, tag=f"hT{i}") for i in range(NIC)]
        qT = [acts.tile([128, TOK], f32r, tag=f"qT{i}") for i in range(NIC)]
        ctxT = [acts.tile([128, TOK], f32r, tag=f"cT{i}") for i in range(NIC)]
        x2T = [acts.tile([128, TOK], f32, tag=f"x2T{i}") for i in range(NIC)]
        h2T = [acts.tile([128, TOK], f32r, tag=f"h2T{i}") for i in range(NIC)]
        yT = [acts.tile([128, TOK], f32, tag=f"yT{i}") for i in range(NIC)]

        kv_in = dram.tile([2 * E, TOK], f32r)
        kv_all = dram.tile([4 * 2 * E, TOK], f32r)

        # ================= LN1 (natural) + transposes =================
        def layernorm_natural(x_nat_tiles, w_tile, b_tile, out_pool, out_tag):
            """x_nat_tiles: 4x [128, E] f32 natural. Returns 4 normalized tiles."""
            outs = []
            for t in range(4):
                xt = x_nat_tiles[t]
                stats = out_pool.tile([128, 2, 6], f32, tag=f"{out_tag}_st")
                nc.vector.bn_stats(out=stats[:, 0, :], in_=xt[:, 0:512])
                nc.vector.bn_stats(out=stats[:, 1, :], in_=xt[:, 512:1024])
                mv = out_pool.tile([128, 2], f32, tag=f"{out_tag}_mv")
                nc.vector.bn_aggr(out=mv[:, :], in_=stats[:, :, :])
                rstd = out_pool.tile([128, 1], f32, tag=f"{out_tag}_rs")
                nc.scalar.activation(out=rstd[:, :], in_=mv[:, 1:2],
                                     func=AF.Sqrt, bias=eps_t[:, :], scale=1.0)
                nc.vector.reciprocal(out=rstd[:, :], in_=rstd[:, :])
                hn = out_pool.tile([128, E], f32, tag=f"{out_tag}_h")
                # (x - mean) * rstd
                nc.vector.tensor_scalar(hn[:, :], xt[:, :], mv[:, 0:1], rstd[:, :],
                                        ALU.subtract, ALU.mult)
                # * w + b
                nc.vector.scalar_tensor_tensor(
                    out=hn[:, :], in0=hn[:, :], scalar=1.0, in1=w_tile[:, :],
                    op0=ALU.bypass, op1=ALU.mult)
                nc.vector.tensor_add(hn[:, :], hn[:, :], b_tile[:, :])
                outs.append(hn)
            return outs

        with (
            tc.tile_pool(name="p1sb", bufs=2) as p1sb,
            tc.tile_pool(name="p1ps", bufs=4, space="PSUM") as p1ps,
        ):
            x_nat = []
            for t in range(4):
                xt = p1sb.tile([128, E], f32, tag="xnat")
                nc.sync.dma_start(out=xt[:, :], in_=x_in[128 * t:128 * (t + 1), :])
                x_nat.append(xt)
            h_nat = layernorm_natural(x_nat, lnw_t[0], lnb_t[0], p1sb, "ln1")
            # transposes: x^T (f32) and h^T (f32r)
            for t in range(4):
                for ic in range(NIC):
                    pt = p1ps.tile([128, 128], f32, tag="tp")
                    nc.tensor.transpose(pt[:, :], x_nat[t][:, 128 * ic:128 * (ic + 1)],
                                        ident[:, :])
                    nc.scalar.copy(out=xT[ic][:, 128 * t:128 * (t + 1)], in_=pt[:, :])
                    pt2 = p1ps.tile([128, 128], f32, tag="tp2")
                    nc.tensor.transpose(pt2[:, :], h_nat[t][:, 128 * ic:128 * (ic + 1)],
                                        ident[:, :])
                    nc.scalar.copy(out=hT[ic][:, 128 * t:128 * (t + 1)], in_=pt2[:, :])

        # ================= qkv^T (24 feature chunks x 8 in-chunks) =========
        with (
            tc.tile_pool(name="p2w", bufs=3) as p2w,
            tc.tile_pool(name="p2sb", bufs=3) as p2sb,
            tc.tile_pool(name="p2ps", bufs=2, space="PSUM") as p2ps,
        ):
            for g in range(6):  # groups of 4 feature chunks
                psums = [p2ps.tile([128, TOK], f32, tag=f"mm{j}") for j in range(4)]
                for ic in range(NIC):
                    wt = p2w.tile([128, 512], f32r, tag="w")
                    nc.sync.dma_start(
                        out=wt[:, :],
                        in_=w_qkv[128 * ic:128 * (ic + 1), 512 * g:512 * (g + 1)])
                    for j in range(4):
                        nc.tensor.matmul(psums[j][:, :], wt[:, 128 * j:128 * (j + 1)],
                                         hT[ic][:, :], start=(ic == 0),
                                         stop=(ic == NIC - 1))
                for j in range(4):
                    fc = 4 * g + j
                    if fc < 8:  # q rows -> resident qT
                        nc.scalar.copy(out=qT[fc][:, :], in_=psums[j][:, :])
                    else:       # k,v rows -> staging, then DRAM for AllGather
                        st = p2sb.tile([128, TOK], f32r, tag="kv")
                        nc.scalar.copy(out=st[:, :], in_=psums[j][:, :])
                        r0 = 128 * (fc - 8)
                        nc.sync.dma_start(out=kv_in[r0:r0 + 128, :], in_=st[:, :])

        # ================= AllGather k,v =================
        nc.gpsimd.collective_compute(
            "AllGather", ALU.bypass, replica_groups=GROUPS,
            ins=[kv_in.opt()], outs=[kv_all.opt()])

        # ================= attention =================
        with (
            tc.tile_pool(name="p3m", bufs=1) as p3m,
            tc.tile_pool(name="p3k", bufs=3) as p3k,
            tc.tile_pool(name="p3p", bufs=2) as p3p,
            tc.tile_pool(name="p3ps", bufs=1, space="PSUM") as p3ctx,
            tc.tile_pool(name="p3ss", bufs=2, space="PSUM") as p3ss,
            tc.tile_pool(name="p3tp", bufs=2, space="PSUM") as p3tp,
        ):
            mask_sb = p3m.tile([128, NKB * TOK], f32)
            for kb in range(NKB):
                nc.sync.dma_start(out=mask_sb[:, TOK * kb:TOK * (kb + 1)],
                                  in_=maskT[128 * kb:128 * (kb + 1), :])
            for hp in range(8):  # head pairs
                cpsA = p3ctx.tile([66, TOK], f32, tag="ctxA")
                cpsB = p3ctx.tile([66, TOK], f32, tag="ctxB")
                for kb in range(NKB):
                    j, ksl = kb // 4, kb % 4
                    kt = p3k.tile([128, 128], f32r, tag="kt")
                    nc.sync.dma_start(
                        out=kt[:, :],
                        in_=kv_all[2048 * j + 128 * hp:2048 * j + 128 * (hp + 1),
                                   128 * ksl:128 * (ksl + 1)])
                    msl = mask_sb[:, TOK * kb:TOK * (kb + 1)]
                    for h in range(2):  # head within pair
                        lo, hi = 64 * h, 64 * (h + 1)
                        sps = p3ss.tile([128, TOK], f32, tag=f"s{h}")
                        nc.tensor.matmul(sps[:, :], kt[lo:hi, :], qT[hp][lo:hi, :],
                                         start=True, stop=True,
                                         tile_position=(lo, 0))
                        nc.vector.tensor_add(sps[:, :], sps[:, :], msl)
                        pT = p3p.tile([128, TOK], f32r, tag=f"p{h}")
                        nc.scalar.activation(out=pT[:, :], in_=sps[:, :],
                                             func=AF.Exp, scale=0.125)
                        # v_aug: [128 keys, 66] with ones in col 64 (65 pad)
                        vt = p3k.tile([64, 128], f32r, tag=f"vt{h}")
                        r0 = 2048 * j + 1024 + 64 * (2 * hp + h)
                        nc.sync.dma_start(
                            out=vt[:, :],
                            in_=kv_all[r0:r0 + 64, 128 * ksl:128 * (ksl + 1)])
                        tp = p3tp.tile([128, 64], f32r, tag=f"tp{h}")
                        nc.tensor.transpose(tp[:, :], vt[:, :], identr[0:64, 0:64])
                        va = p3k.tile([128, 66], f32r, tag=f"va{h}")
                        nc.scalar.copy(out=va[:, 0:64], in_=tp[:, :])
                        nc.vector.memset(va[:, 64:66], 1.0)
                        cps = cpsA if h == 0 else cpsB
                        nc.tensor.matmul(cps[:, :], va[:, :], pT[:, :],
                                         start=(kb == 0), stop=(kb == NKB - 1))
                # normalize: rows 0:64 / row 64, write into ctxT[hp].
                # partition-broadcast of 1/sums via K=1 matmul with a ones col.
                for h, cps in ((0, cpsA), (1, cpsB)):
                    rec = p3p.tile([1, TOK], f32, tag=f"r{h}")
                    nc.vector.reciprocal(out=rec[:, :], in_=cps[64:65, :])
                    rec_r = p3p.tile([1, TOK], f32r, tag=f"rr{h}")
                    nc.scalar.copy(out=rec_r[:, :], in_=rec[:, :])
                    rbp = p3tp.tile([64, TOK], f32, tag=f"rb{h}")
                    nc.tensor.matmul(rbp[:, :], ones_r[:, :], rec_r[:, :],
                                     start=True, stop=True)
                    rb = p3p.tile([64, TOK], f32, tag=f"rbs{h}")
                    nc.scalar.copy(out=rb[:, :], in_=rbp[:, :])
                    nc.vector.tensor_tensor(
                        out=ctxT[hp][64 * h:64 * (h + 1), :],
                        in0=cps[0:64, :], in1=rb[:, :], op=ALU.mult)

        # ================= attn_proj + bias + residual =================
        with (
            tc.tile_pool(name="p4w", bufs=3) as p4w,
            tc.tile_pool(name="p4ps", bufs=2, space="PSUM") as p4ps,
        ):
            for g in range(2):
                psums = [p4ps.tile([128, TOK], f32, tag=f"mm{j}") for j in range(4)]
                for ic in range(NIC):
                    wt = p4w.tile([128, 512], f32r, tag="w")
                    nc.sync.dma_start(
                        out=wt[:, :],
                        in_=w_ap[128 * ic:128 * (ic + 1), 512 * g:512 * (g + 1)])
                    for j in range(4):
                        nc.tensor.matmul(psums[j][:, :], wt[:, 128 * j:128 * (j + 1)],
                                         ctxT[ic][:, :], start=(ic == 0),
                                         stop=(ic == NIC - 1))
                for j in range(4):
                    oc = 4 * g + j
                    # x2^T = (psum + b_ap[oc]) + x^T
                    nc.vector.scalar_tensor_tensor(
                        out=x2T[oc][:, :], in0=psums[j][:, :],
                        scalar=bap_t[:, oc:oc + 1], in1=xT[oc][:, :],
                        op0=ALU.add, op1=ALU.add)

        # ================= LN2 (transpose -> natural -> back) ==========
        with (
            tc.tile_pool(name="p5sb", bufs=2) as p5sb,
            tc.tile_pool(name="p5ps", bufs=4, space="PSUM") as p5ps,
        ):
            x2_nat = []
            for t in range(4):
                xt = p5sb.tile([128, E], f32, tag="x2nat")
                for ic in range(NIC):
                    pt = p5ps.tile([128, 128], f32, tag="tp")
                    nc.tensor.transpose(pt[:, :], x2T[ic][:, 128 * t:128 * (t + 1)],
                                        ident[:, :])
                    nc.scalar.copy(out=xt[:, 128 * ic:128 * (ic + 1)], in_=pt[:, :])
                x2_nat.append(xt)
            h2_nat = layernorm_natural(x2_nat, lnw_t[1], lnb_t[1], p5sb, "ln2")
            for t in range(4):
                for ic in range(NIC):
                    pt = p5ps.tile([128, 128], f32, tag="tp2")
                    nc.tensor.transpose(pt[:, :], h2_nat[t][:, 128 * ic:128 * (ic + 1)],
                                        ident[:, :])
                    nc.scalar.copy(out=h2T[ic][:, 128 * t:128 * (t + 1)], in_=pt[:, :])

        # ================= MLP =================
        with (
            tc.tile_pool(name="p6w", bufs=3) as p6w,
            tc.tile_pool(name="p6ps", bufs=2, space="PSUM") as p6ps,
        ):
            # reuse dead activation tag slots (hT/qT/cT/xT) for the 8MB g^T
            g_tags = ([f"hT{i}" for i in range(8)] + [f"qT{i}" for i in range(8)]
                      + [f"cT{i}" for i in range(8)] + [f"xT{i}" for i in range(8)])
            gT = [acts.tile([128, TOK], f32r, tag=g_tags[i]) for i in range(32)]
            for g in range(8):  # fc: groups of 4 hidden chunks
                psums = [p6ps.tile([128, TOK], f32, tag=f"mm{j}") for j in range(4)]
                for ic in range(NIC):
                    wt = p6w.tile([128, 512], f32r, tag="w")
                    nc.sync.dma_start(
                        out=wt[:, :],
                        in_=w_fc[128 * ic:128 * (ic + 1), 512 * g:512 * (g + 1)])
                    for j in range(4):
                        nc.tensor.matmul(psums[j][:, :], wt[:, 128 * j:128 * (j + 1)],
                                         h2T[ic][:, :], start=(ic == 0),
                                         stop=(ic == NIC - 1))
                for j in range(4):
                    hc = 4 * g + j
                    nc.scalar.activation(out=gT[hc][:, :], in_=psums[j][:, :],
                                         func=AF.Gelu_apprx_tanh,
                                         bias=bfc_t[:, hc:hc + 1], scale=1.0)
            for g in range(2):  # proj
                psums = [p6ps.tile([128, TOK], f32, tag=f"pm{j}") for j in range(4)]
                for ic in range(32):
                    wt = p6w.tile([128, 512], f32r, tag="w2")
                    nc.sync.dma_start(
                        out=wt[:, :],
                        in_=w_pr[128 * ic:128 * (ic + 1), 512 * g:512 * (g + 1)])
                    for j in range(4):
                        nc.tensor.matmul(psums[j][:, :], wt[:, 128 * j:128 * (j + 1)],
                                         gT[ic][:, :], start=(ic == 0),
                                         stop=(ic == 31))
                for j in range(4):
                    oc = 4 * g + j
                    nc.vector.scalar_tensor_tensor(
                        out=yT[oc][:, :], in0=psums[j][:, :],
                        scalar=bpr_t[:, oc:oc + 1], in1=x2T[oc][:, :],
                        op0=ALU.add, op1=ALU.add)

        # ================= output transpose + store =================
        with (
            tc.tile_pool(name="p7sb", bufs=2) as p7sb,
            tc.tile_pool(name="p7ps", bufs=4, space="PSUM") as p7ps,
        ):
            for t in range(4):
                yt = p7sb.tile([128, E], f32, tag="ynat")
                for ic in range(NIC):
                    pt = p7ps.tile([128, 128], f32, tag="tp")
                    nc.tensor.transpose(pt[:, :], yT[ic][:, 128 * t:128 * (t + 1)],
                                        ident[:, :])
                    nc.scalar.copy(out=yt[:, 128 * ic:128 * (ic + 1)], in_=pt[:, :])
                nc.sync.dma_start(out=y_out[128 * t:128 * (t + 1), :], in_=yt[:, :])

        dram.release()
        acts.release()
        consts.release()

    nc.compile()
    return nc


def make_in_maps(inputs):
    """inputs: the reference setup_inputs() dict. Returns list of 8 per-core maps."""
    x = np.asarray(inputs["x"], np.float32)
    w_qkv = np.asarray(inputs["w_qkv"], np.float32)
    w_ap = np.asarray(inputs["w_attn_proj"], np.float32)
    w_fc = np.asarray(inputs["w_fc"], np.float32)
    w_pr = np.asarray(inputs["w_proj"], np.float32)
    b_ap = np.asarray(inputs["b_attn_proj"], np.float32).reshape(8, 128).T.copy()
    b_fc = np.asarray(inputs["b_fc"], np.float32).reshape(32, 128).T.copy()
    b_pr = np.asarray(inputs["b_proj"], np.float32).reshape(8, 128).T.copy()
    ln1w = np.asarray(inputs["ln1_w"], np.float32)
    ln1b = np.asarray(inputs["ln1_b"], np.float32)
    ln2w = np.asarray(inputs["ln2_w"], np.float32)
    ln2b = np.asarray(inputs["ln2_b"], np.float32)

    in_maps = []
    kpos = np.arange(KEYS)[:, None]
    for c in range(N_CORES):
        b, o = c // 4, c % 4
        qpos = (TOK * o + np.arange(TOK))[None, :]
        mask = np.where(qpos >= kpos, 0.0, NEG).astype(np.float32)
        in_maps.append({
            "x": x[b, TOK * o:TOK * (o + 1), :],
            "maskT": mask,
            "w_qkv": w_qkv, "w_ap": w_ap, "w_fc": w_fc, "w_pr": w_pr,
            "b_ap": b_ap, "b_fc": b_fc, "b_pr": b_pr,
            "ln1w": ln1w, "ln1b": ln1b, "ln2w": ln2w, "ln2b": ln2b,
        })
    return in_maps


def assemble(results):
    y = np.zeros((2, KEYS, E), np.float32)
    for c in range(N_CORES):
        b, o = c // 4, c % 4
        y[b, TOK * o:TOK * (o + 1), :] = results[c]["y"]
    return y


# ======================================================================
# Harness entry point: full inputs -> full output, 8-core SPMD inside.
# ======================================================================
_CACHE = {}


def _get_runner():
    """Build the Bass program once and wrap it in a reusable sharded jit."""
    import jax
    from jax.sharding import Mesh, PartitionSpec
    from jax.experimental.shard_map import shard_map
    from concourse.bass2jax import (_bass_exec_p, install_neuronx_cc_hook,
                                    partition_id_tensor)

    nc = build(gelu_mode="hw")
    install_neuronx_cc_hook()

    partition_name = nc.partition_id_tensor.name if nc.partition_id_tensor else None
    in_names, out_names, out_avals, zero_outs = [], [], [], []
    for alloc in nc.m.functions[0].allocations:
        if not isinstance(alloc, mybir.MemoryLocationSet):
            continue
        name = alloc.memorylocations[0].name
        if alloc.kind == "ExternalInput":
            if name != partition_name:
                in_names.append(name)
        elif alloc.kind == "ExternalOutput":
            out_names.append(name)
            shape = tuple(alloc.tensor_shape)
            dtype = mybir.dt.np(alloc.dtype)
            out_avals.append(jax.core.ShapedArray(shape, dtype))
            zero_outs.append(np.zeros(shape, dtype))
    n_params, n_outs = len(in_names), len(out_avals)
    all_in = in_names + out_names + ([partition_name] if partition_name else [])
    donate = tuple(range(n_params, n_params + n_outs))

    def _body(*args):
        operands = list(args)
        if partition_name is not None:
            operands.append(partition_id_tensor())
        return tuple(_bass_exec_p.bind(
            *operands, out_avals=tuple(out_avals), in_names=tuple(all_in),
            out_names=tuple(out_names), lowering_input_output_aliases=(),
            sim_require_finite=True, sim_require_nnan=True, nc=nc))

    devices = jax.devices()[:N_CORES]
    mesh = Mesh(np.asarray(devices), ("core",))
    sharded = jax.jit(
        shard_map(_body, mesh=mesh,
                  in_specs=(PartitionSpec("core"),) * (n_params + n_outs),
                  out_specs=(PartitionSpec("core"),) * n_outs,
                  check_rep=False),
        donate_argnums=donate, keep_unused=True)
    return nc, sharded, in_names, out_names, out_avals, zero_outs


def _run_fast(in_maps):
    import jax
    nc, sharded, in_names, out_names, out_avals, zero_outs = _CACHE["rt"]
    concat_in = [
        np.concatenate([np.asarray(in_maps[c][nm]) for c in range(N_CORES)], axis=0)
        for nm in in_names]
    zs = [np.zeros((N_CORES * z.shape[0], *z.shape[1:]), z.dtype)
          for z in zero_outs]
    outs = sharded(*concat_in, *zs)
    jax.block_until_ready(outs)
    return [
        {nm: np.asarray(outs[i]).reshape(N_CORES, *out_avals[i].shape)[c]
         for i, nm in enumerate(out_names)}
        for c in range(N_CORES)]


def kernel(**inputs):
    """Decoder block on 8 trn2 NeuronCores.

    Takes the full (unsharded) reference inputs, returns [2, 2048, 1024] f32.
    """
    in_maps = make_in_maps(inputs)

    last_err = None
    for _attempt in range(3):
        try:
            if "rt" not in _CACHE:
                _CACHE["rt"] = _get_runner()
            return assemble(_run_fast(in_maps))
        except Exception as e:  # wedged-device flake or jit issue: retry
            last_err = e
            _CACHE.pop("rt", None)

    # last resort: plain one-shot SPMD path
    from concourse.bass_utils import run_bass_kernel_spmd
    try:
        nc = build(gelu_mode="hw")
        res = run_bass_kernel_spmd(nc, in_maps, list(range(N_CORES)))
        return assemble([res.results[i] for i in range(N_CORES)])
    except Exception:
        raise last_err
